# revision 17
# baseline (speedup 1.0000x reference)
"""GCN link-prediction (2x GCNConv + edge decode) on 8 Trainium2 NeuronCores.

Sharding: nodes (and their incident in-edges) are partitioned across the 8
cores by destination node.  Each core does local gather -> onehot-matmul
segment-sum -> scale for its destination shard; node-feature tables are
exchanged between layers with AllGather collectives.  The small 128x128
weight matrices are replicated.
"""

import sys

sys.path.insert(0, "/opt/trn_rl_repo")

import numpy as np
import ml_dtypes

import concourse.bacc as bacc
import concourse.bass as bass
import concourse.tile as tile
import concourse.mybir as mybir
from concourse import bass_utils

DT = mybir.dt
BF16 = ml_dtypes.bfloat16
NCORES = 8
CB = 4  # dst blocks per gather chunk


def _ceil(a, b):
    return -(-a // b)


def _wrap16(idx):
    """[n] int -> [128, n//16] int16, wrapped in 16 partitions, replicated x8."""
    n = len(idx)
    assert n % 16 == 0
    w = np.ascontiguousarray(idx.reshape(n // 16, 16).T.astype(np.int16))
    return np.tile(w, (8, 1))


class Plan:
    pass


def _build_plan(N, E, ED):
    """Static structure shared by all cores (shapes baked into the program)."""
    p = Plan()
    assert N % NCORES == 0
    p.N = N
    p.shard = N // NCORES
    p.nblk = _ceil(p.shard, 128)
    p.qsize = _ceil(N, 4 * 128) * 128
    assert p.qsize <= 32767
    p.qrows = [min(p.qsize, N - q * p.qsize) for q in range(4)]
    p.ntg = _ceil(N, 128)  # global node tiles
    p.E = E
    p.ED = ED
    p.dec_per_core = ED // NCORES
    assert ED % NCORES == 0
    p.chunks = [list(range(b, min(b + CB, p.nblk))) for b in range(0, p.nblk, CB)]
    return p


def _prep_layer_edges(p, src, dst):
    """Group layer edges per core by (dst block, src quarter); pad each group
    to a common (across cores) multiple of 128.  Returns per-core idx/dstrel
    streams plus the static call/tile schedule."""
    N = p.N
    core = dst // p.shard
    blk = (dst % p.shard) // 128
    dstrel = (dst % p.shard) % 128
    q = np.minimum(src // p.qsize, 3)
    srcrel = src - q * p.qsize

    counts = np.zeros((NCORES, p.nblk, 4), np.int64)
    np.add.at(counts, (core, blk, q), 1)
    pad = _ceil(np.maximum(counts.max(0), 1), 128) * 128  # [nblk, 4]
    # order edges per (core, blk, q)
    order = np.lexsort((dstrel, q, blk, core))
    so_core, so_blk, so_q = core[order], blk[order], q[order]
    so_srcrel, so_dstrel = srcrel[order], dstrel[order]
    # group start offsets in sorted stream per (core, blk, q)
    grp_of = (so_core * p.nblk + so_blk) * 4 + so_q
    ngrp = NCORES * p.nblk * 4
    starts = np.searchsorted(grp_of, np.arange(ngrp))
    ends = np.searchsorted(grp_of, np.arange(ngrp) + 1)

    # schedule: chunk -> q -> blk -> tiles; calls split to <=1024 idx
    # (Q7 dma_gather scratch caps num_idxs per call at 1024)
    MAXT = 8
    calls = []  # dicts: ci, q, col0, n_idx, tiles [(blk, tile_id, stop)]
    NT = 0
    col = 0
    tile_meta = []  # (blk, q, k) per tile in stream order
    for ci, ch in enumerate(p.chunks):
        for qq in range(4):
            if int(sum(pad[b, qq] for b in ch)) == 0:
                continue
            tiles = []
            for b in ch:
                for k in range(pad[b, qq] // 128):
                    tiles.append([b, NT, False])
                    tile_meta.append((b, qq))
                    NT += 1
            for i in range(0, len(tiles), MAXT):
                piece = tiles[i : i + MAXT]
                n_idx = 128 * len(piece)
                calls.append(dict(ci=ci, q=qq, col0=col, n_idx=n_idx,
                                  tiles=piece))
                col += n_idx // 16
    # stop flags: last tile of each block in stream order
    last_tile = {}
    for t, (b, qq) in enumerate(tile_meta):
        last_tile[b] = t
    for c in calls:
        for rec in c["tiles"]:
            if last_tile[rec[0]] == rec[1]:
                rec[2] = True
    p.l_calls = calls
    p.l_NT = NT
    p.l_cols = col

    # per-core streams
    idx_arrs, dst_arrs = [], []
    for cc in range(NCORES):
        idx_s = np.zeros(NT * 128, np.int64)
        dst_s = np.full(NT * 128, -1.0, np.float32)
        t = 0
        for ch in p.chunks:
            for qq in range(4):
                if sum(pad[b, qq] for b in ch) == 0:
                    continue
                for b in ch:
                    g = (cc * p.nblk + b) * 4 + qq
                    s, e = starts[g], ends[g]
                    npad = pad[b, qq]
                    o = t * 128
                    idx_s[o : o + (e - s)] = so_srcrel[s:e]
                    dst_s[o : o + (e - s)] = so_dstrel[s:e]
                    t += npad // 128
        # idx stream laid out per call (wrapped16 within each call)
        idx_cols = np.zeros((128, col), np.int16)
        for c in calls:
            t0 = min(r[1] for r in c["tiles"])
            seg = idx_s[t0 * 128 : t0 * 128 + c["n_idx"]]
            idx_cols[:, c["col0"] : c["col0"] + c["n_idx"] // 16] = _wrap16(seg)
        idx_arrs.append(idx_cols)
        dst_arrs.append(
            np.ascontiguousarray(dst_s.reshape(NT, 128).T)
        )
    return idx_arrs, dst_arrs


def _prep_decode(p, e0, e1):
    """Bucket decode edges per core by (quarter(e0), quarter(e1))."""
    ED = p.ED
    eid = np.arange(ED)
    core = eid // p.dec_per_core
    qa = np.minimum(e0 // p.qsize, 3)
    qb = np.minimum(e1 // p.qsize, 3)
    bkt = qa * 4 + qb
    counts = np.zeros((NCORES, 16), np.int64)
    np.add.at(counts, (core, bkt), 1)
    bpad = _ceil(np.maximum(counts.max(0), 1), 512) * 512  # [16]
    p.d_bpad = bpad
    p.d_nchunk = int(bpad.sum()) // 512
    # chunk schedule: (bucket, k within bucket); gathers in 1024-idx pieces
    chunks = []
    for b in range(16):
        for k in range(bpad[b] // 512):
            chunks.append(dict(bkt=b, k=k))
    p.d_chunks = chunks
    p.d_cols0 = np.concatenate([[0], np.cumsum(bpad // 16)])  # e0 idx col offsets
    CD = int(bpad.sum() // 16)
    p.d_CD = CD

    order = np.lexsort((eid, bkt, core))
    so_core, so_bkt = core[order], bkt[order]
    so_e0, so_e1, so_eid = e0[order], e1[order], eid[order]
    so_qa, so_qb = qa[order], qb[order]
    grp_of = so_core * 16 + so_bkt
    starts = np.searchsorted(grp_of, np.arange(NCORES * 16))
    ends = np.searchsorted(grp_of, np.arange(NCORES * 16) + 1)

    idx0_arrs, idx1_arrs, emaps = [], [], []
    for cc in range(NCORES):
        i0 = np.zeros(int(bpad.sum()), np.int64)
        i1 = np.zeros(int(bpad.sum()), np.int64)
        em = np.full(int(bpad.sum()), -1, np.int64)
        o = 0
        for b in range(16):
            g = cc * 16 + b
            s, e = starts[g], ends[g]
            i0[o : o + (e - s)] = so_e0[s:e] - so_qa[s:e] * p.qsize
            i1[o : o + (e - s)] = so_e1[s:e] - so_qb[s:e] * p.qsize
            em[o : o + (e - s)] = so_eid[s:e]
            o += bpad[b]
        c0 = np.zeros((128, CD), np.int16)
        c1 = np.zeros((128, CD), np.int16)
        for b in range(16):
            lo, hi = int(bpad[:b].sum()), int(bpad[: b + 1].sum())
            c0[:, lo // 16 : hi // 16] = _wrap16(i0[lo:hi])
            c1[:, lo // 16 : hi // 16] = _wrap16(i1[lo:hi])
        idx0_arrs.append(c0)
        idx1_arrs.append(c1)
        emaps.append(em)
    return idx0_arrs, idx1_arrs, emaps


def _build_bass(p):
    nc = bacc.Bacc("TRN2", target_bir_lowering=False, debug=False,
                   num_devices=NCORES, num_swdge_queues=4)
    N, nblk, ntg = p.N, p.nblk, p.ntg
    NT, COLS = p.l_NT, p.l_cols

    xT = nc.dram_tensor("xT", [128, N], DT.bfloat16, kind="ExternalInput")
    w1 = nc.dram_tensor("w1", [128, 128], DT.float32, kind="ExternalInput")
    w2 = nc.dram_tensor("w2", [128, 128], DT.float32, kind="ExternalInput")
    wcat = nc.dram_tensor("wcat", [128, 4], DT.float32, kind="ExternalInput")
    iota_d = nc.dram_tensor("iota", [128, 128], DT.bfloat16, kind="ExternalInput")
    ident_d = nc.dram_tensor("ident", [128, 128], DT.bfloat16, kind="ExternalInput")
    degg_d = nc.dram_tensor("degg", [128, ntg], DT.float32, kind="ExternalInput")
    degb_d = nc.dram_tensor("degb", [128, nblk], DT.float32, kind="ExternalInput")
    idxl_d = nc.dram_tensor("idxl", [128, COLS], DT.int16, kind="ExternalInput")
    dstl_d = nc.dram_tensor("dstl", [128, NT], DT.float32, kind="ExternalInput")
    idx0_d = nc.dram_tensor("idx0", [128, p.d_CD], DT.int16, kind="ExternalInput")
    idx1_d = nc.dram_tensor("idx1", [128, p.d_CD], DT.int16, kind="ExternalInput")
    out_d = nc.dram_tensor("out", [p.d_nchunk * 2, 512], DT.float32,
                           kind="ExternalOutput")

    h1q = [
        nc.dram_tensor(f"h1q{q}", [p.qrows[q], 128], DT.bfloat16, kind="Internal")
        for q in range(4)
    ]
    bounce1 = nc.dram_tensor("bounce1", [p.shard, 128], DT.bfloat16, kind="Internal")
    h2full = nc.dram_tensor("h2full", [N, 128], DT.bfloat16, kind="Internal",
                            addr_space="Shared")
    bounce2 = nc.dram_tensor("bounce2", [p.shard, 128], DT.bfloat16, kind="Internal")
    z2full = nc.dram_tensor("z2full", [N, 128], DT.bfloat16, kind="Internal",
                            addr_space="Shared")

    AF = mybir.ActivationFunctionType
    OP = mybir.AluOpType
    qrot = [0]

    with tile.TileContext(nc) as tc:
        with tc.tile_pool(name="const", bufs=1) as cp:
            iota_t = cp.tile([128, 128], DT.bfloat16)
            nc.sync.dma_start(iota_t[:], iota_d[:])
            ident_t = cp.tile([128, 128], DT.bfloat16)
            nc.sync.dma_start(ident_t[:], ident_d[:])
            zero_t = cp.tile([128, 512], DT.bfloat16)
            nc.vector.memset(zero_t[:], 0.0)
            w1f = cp.tile([128, 128], DT.float32, name="w1f")
            nc.sync.dma_start(w1f[:], w1[:])
            w1b = cp.tile([128, 128], DT.bfloat16, name="w1b")
            nc.scalar.copy(w1b[:], w1f[:])
            w2f = cp.tile([128, 128], DT.float32, name="w2f")
            nc.sync.dma_start(w2f[:], w2[:])
            w2b = cp.tile([128, 128], DT.bfloat16, name="w2b")
            nc.scalar.copy(w2b[:], w2f[:])
            wcf = cp.tile([128, 4], DT.float32, name="wcf")
            nc.sync.dma_start(wcf[:], wcat[:])
            wcb = cp.tile([128, 4], DT.bfloat16, name="wcb")
            nc.scalar.copy(wcb[:], wcf[:])

            # dinv = deg > 0 ? 1/sqrt(deg) : 0   (both layouts)
            def make_dinv(deg_d, w, nm):
                deg = cp.tile([128, w], DT.float32, name=f"deg{nm}")
                nc.sync.dma_start(deg[:], deg_d[:])
                m = cp.tile([128, w], DT.float32, name=f"m{nm}")
                nc.vector.tensor_scalar(m[:], deg[:], 0.0, None, op0=OP.is_gt)
                t1 = cp.tile([128, w], DT.float32, name=f"t1{nm}")
                nc.vector.tensor_tensor(t1[:], deg[:], m[:], op=OP.subtract)
                nc.vector.tensor_scalar_add(t1[:], t1[:], 1.0)
                nc.scalar.sqrt(t1[:], t1[:])
                r = cp.tile([128, w], DT.float32, name=f"r{nm}")
                nc.vector.reciprocal(r[:], t1[:])
                dv = cp.tile([128, w], DT.float32, name=f"dinv{nm}")
                nc.vector.tensor_tensor(dv[:], r[:], m[:], op=OP.mult)
                return dv

            dinvg = make_dinv(degg_d, ntg, "g")
            dinvb = make_dinv(degb_d, nblk, "b")

            idxl_t = cp.tile([128, COLS], DT.int16)
            nc.sync.dma_start(idxl_t[:], idxl_d[:])
            dstl_t = cp.tile([128, NT], DT.float32)
            nc.sync.dma_start(dstl_t[:], dstl_d[:])

            z1r = cp.tile([128, nblk * 128], DT.bfloat16, name="z1r")

            # ---- L1 node table: h1' = dinv * (x @ W1), replicated ----
            qtiles = _ceil(p.qsize, 128)
            with tc.tile_pool(name="xtp", bufs=4) as xp, \
                 tc.tile_pool(name="tps", bufs=4, space="PSUM") as tps, \
                 tc.tile_pool(name="hop", bufs=4) as hop:
                for t in range(ntg):
                    rt = min(128, N - t * 128)
                    q = t // qtiles
                    lr = (t - q * qtiles) * 128
                    xt = xp.tile([128, 128], DT.bfloat16, tag="xt")
                    nc.sync.dma_start(xt[:, 0:rt], xT[:, t * 128 : t * 128 + rt])
                    ps = tps.tile([128, 128], DT.float32, tag="ps")
                    nc.tensor.matmul(ps[0:rt, :], xt[:, 0:rt], w1b[:],
                                     start=True, stop=True)
                    ho = hop.tile([128, 128], DT.bfloat16, tag="ho")
                    nc.scalar.activation(ho[0:rt, :], ps[0:rt, :], AF.Copy,
                                         scale=dinvg[0:rt, t : t + 1])
                    nc.sync.dma_start(h1q[q][lr : lr + rt, :], ho[0:rt, :])

            # ---- layer message passing ----
            def layer_pass(table_aps, readout):
                with tc.tile_pool(name="gt", bufs=6) as gp, \
                     tc.tile_pool(name="ohp", bufs=6) as ohp, \
                     tc.tile_pool(name="lps", bufs=8, space="PSUM") as lps:
                    pstiles = {}
                    for ci, ch in enumerate(p.chunks):
                        for b in ch:
                            ps = lps.tile([128, 128], DT.float32, tag="zps",
                                          name=f"zps{b}")
                            nc.tensor.matmul(ps[:], zero_t[:, 0:128],
                                             zero_t[:, 0:128],
                                             start=True, stop=False)
                            pstiles[b] = ps
                        for c in p.l_calls:
                            if c["ci"] != ci:
                                continue
                            nt = c["n_idx"] // 128
                            gt = gp.tile([128, nt, 128], DT.bfloat16, tag="gt")
                            nc.gpsimd.dma_gather(
                                gt[:], table_aps[c["q"]],
                                idxl_t[:, c["col0"] : c["col0"] + c["n_idx"] // 16],
                                c["n_idx"], c["n_idx"], 128,
                                queue_num=qrot[0] % 4,
                            )
                            qrot[0] += 1
                            t0 = c["tiles"][0][1]
                            for b, t, stp in c["tiles"]:
                                oh = ohp.tile([128, 128], DT.bfloat16, tag="oh")
                                nc.vector.tensor_scalar(
                                    oh[:], iota_t[:], dstl_t[:, t : t + 1], None,
                                    op0=OP.is_equal,
                                )
                                nc.tensor.matmul(pstiles[b][:], oh[:],
                                                 gt[:, t - t0, :],
                                                 start=False, stop=stp)
                        for b in ch:
                            readout(b, pstiles[b])

            # L1: z1r = relu(dinv * z);  keep in SBUF
            def l1_read(b, ps):
                nc.scalar.activation(
                    z1r[:, b * 128 : (b + 1) * 128], ps[:], AF.Relu,
                    scale=dinvb[:, b : b + 1],
                )

            layer_pass([h1q[q][:] for q in range(4)], l1_read)

            # ---- L2 node table: h2' = dinv * (z1r @ W2) -> AllGather ----
            with tc.tile_pool(name="t2ps", bufs=4, space="PSUM") as t2ps, \
                 tc.tile_pool(name="t2sb", bufs=4) as t2sb:
                for b in range(nblk):
                    rt = min(128, p.shard - b * 128)
                    zt_ps = t2ps.tile([128, 128], DT.bfloat16, tag="ztps")
                    nc.tensor.transpose(zt_ps[:],
                                        z1r[:, b * 128 : (b + 1) * 128],
                                        ident_t[:])
                    zt = t2sb.tile([128, 128], DT.bfloat16, tag="zt")
                    nc.scalar.copy(zt[:], zt_ps[:])
                    h2ps = t2ps.tile([128, 128], DT.float32, tag="h2ps")
                    nc.tensor.matmul(h2ps[:], zt[:], w2b[:], start=True, stop=True)
                    h2o = t2sb.tile([128, 128], DT.bfloat16, tag="h2o")
                    nc.scalar.activation(h2o[:], h2ps[:], AF.Copy,
                                         scale=dinvb[:, b : b + 1])
                    nc.sync.dma_start(bounce1[b * 128 : b * 128 + rt, :],
                                      h2o[0:rt, :])
            nc.gpsimd.collective_compute(
                "AllGather", OP.bypass,
                replica_groups=[list(range(NCORES))],
                ins=[bounce1[:]], outs=[h2full[:]],
            )

            h2aps = [
                h2full[q * p.qsize : q * p.qsize + p.qrows[q], :] for q in range(4)
            ]

            # L2: z2 = dinv * z -> bounce2
            with tc.tile_pool(name="z2sb", bufs=4) as z2sb:
                def l2_read(b, ps):
                    rt = min(128, p.shard - b * 128)
                    z2o = z2sb.tile([128, 128], DT.bfloat16, tag="z2o")
                    nc.scalar.activation(z2o[:], ps[:], AF.Copy,
                                         scale=dinvb[:, b : b + 1])
                    nc.sync.dma_start(bounce2[b * 128 : b * 128 + rt, :],
                                      z2o[0:rt, :])

                layer_pass(h2aps, l2_read)
            nc.gpsimd.collective_compute(
                "AllGather", OP.bypass,
                replica_groups=[list(range(NCORES))],
                ins=[bounce2[:]], outs=[z2full[:]],
            )

            z2aps = [
                z2full[q * p.qsize : q * p.qsize + p.qrows[q], :] for q in range(4)
            ]

            # ---- decode: out[e] = z2[e0] @ Wt + z2[e1] @ Wb ----
            with tc.tile_pool(name="di", bufs=1) as dip, \
                 tc.tile_pool(name="dg", bufs=6) as dgp, \
                 tc.tile_pool(name="dps", bufs=2, space="PSUM") as dps, \
                 tc.tile_pool(name="dos", bufs=2) as dos:
                idx0_t = dip.tile([128, p.d_CD], DT.int16)
                nc.sync.dma_start(idx0_t[:], idx0_d[:])
                idx1_t = dip.tile([128, p.d_CD], DT.int16)
                nc.sync.dma_start(idx1_t[:], idx1_d[:])

                gts = {}

                def get_gt(bkt, piece):
                    key = (bkt, piece)
                    if key in gts:
                        return gts[key]
                    lo = int(p.d_bpad[:bkt].sum()) + piece * 512
                    g0 = dgp.tile([128, 1, 512], DT.bfloat16, tag="g0")
                    nc.gpsimd.dma_gather(
                        g0[:], z2aps[bkt // 4],
                        idx0_t[:, lo // 16 : lo // 16 + 32],
                        512, 512, 128, transpose=True,
                        queue_num=qrot[0] % 4,
                    )
                    qrot[0] += 1
                    g1 = dgp.tile([128, 1, 512], DT.bfloat16, tag="g1")
                    nc.gpsimd.dma_gather(
                        g1[:], z2aps[bkt % 4],
                        idx1_t[:, lo // 16 : lo // 16 + 32],
                        512, 512, 128, transpose=True,
                        queue_num=qrot[0] % 4,
                    )
                    qrot[0] += 1
                    gts[key] = (g0, g1)
                    return gts[key]

                for g, c in enumerate(p.d_chunks):
                    g0, g1 = get_gt(c["bkt"], c["k"])
                    k0 = 0
                    ps = dps.tile([2, 512], DT.float32, tag="dpack")
                    nc.tensor.matmul(ps[:], wcb[:, 0:2],
                                     g0[:, 0, k0 : k0 + 512],
                                     start=True, stop=False)
                    nc.tensor.matmul(ps[:], wcb[:, 2:4],
                                     g1[:, 0, k0 : k0 + 512],
                                     start=False, stop=True)
                    del gts[(c["bkt"], c["k"])]
                    po = dos.tile([2, 512], DT.float32, tag="po")
                    if g % 2 == 0:
                        nc.scalar.copy(po[:], ps[:])
                    else:
                        nc.vector.tensor_copy(po[:], ps[:])
                    nc.sync.dma_start(out_d[2 * g : 2 * g + 2, :], po[:])

    nc.compile()
    return nc


def _host_prep(x, W1, W2, Wlin, edge_index, pos_edge_index, neg_edge_index):
    N = x.shape[0]
    src = edge_index[0].astype(np.int64)
    dst = edge_index[1].astype(np.int64)
    e0 = np.concatenate([pos_edge_index[0], neg_edge_index[0]]).astype(np.int64)
    e1 = np.concatenate([pos_edge_index[1], neg_edge_index[1]]).astype(np.int64)
    p = _build_plan(N, src.shape[0], e0.shape[0])

    idxl, dstl = _prep_layer_edges(p, src, dst)
    idx0, idx1, emaps = _prep_decode(p, e0, e1)

    deg = np.bincount(dst, minlength=N).astype(np.float32)
    degg = np.zeros(p.ntg * 128, np.float32)
    degg[:N] = deg
    degg = np.ascontiguousarray(degg.reshape(p.ntg, 128).T)  # [128, ntg]

    xTb = np.ascontiguousarray(x.astype(np.float32).T).astype(BF16)  # [128, N]
    iota = np.tile(np.arange(128, dtype=np.float32), (128, 1)).astype(BF16)
    ident = np.eye(128, dtype=np.float32).astype(BF16)
    # wcat[:, 0:2] = Wlin[:128] (Wt), wcat[:, 2:4] = Wlin[128:] (Wb)
    wcat = np.concatenate(
        [Wlin[:128].astype(np.float32), Wlin[128:].astype(np.float32)], axis=1
    )

    in_maps = []
    for c in range(NCORES):
        degb = np.zeros(p.nblk * 128, np.float32)
        lo = c * p.shard
        degb[: p.shard] = deg[lo : lo + p.shard]
        degb = np.ascontiguousarray(degb.reshape(p.nblk, 128).T)
        in_maps.append({
            "xT": xTb,
            "w1": W1.astype(np.float32),
            "w2": W2.astype(np.float32),
            "wcat": wcat.astype(np.float32),
            "iota": iota,
            "ident": ident,
            "degg": degg,
            "degb": degb,
            "idxl": idxl[c],
            "dstl": dstl[c],
            "idx0": idx0[c],
            "idx1": idx1[c],
        })
    return p, in_maps, emaps


def _assemble(p, results, emaps):
    out = np.zeros((p.ED, 2), np.float32)
    npos = int(p.d_bpad.sum())
    pos = np.arange(npos)
    g = pos // 512
    j = pos % 512
    rows = 2 * g
    for c in range(NCORES):
        arr = results[c]["out"]  # [nchunk*2, 512]
        em = emaps[c]
        valid = em >= 0
        out[em[valid], 0] = arr[rows[valid], j[valid]]
        out[em[valid], 1] = arr[rows[valid] + 1, j[valid]]
    return out


LAST_RESULT = None


def kernel(x, W1, W2, Wlin, edge_index, pos_edge_index, neg_edge_index):
    global LAST_RESULT
    p, in_maps, emaps = _host_prep(
        x, W1, W2, Wlin, edge_index, pos_edge_index, neg_edge_index
    )
    nc = _build_bass(p)
    res = bass_utils.run_bass_kernel_spmd(
        nc, in_maps, core_ids=list(range(NCORES))
    )
    LAST_RESULT = res
    return _assemble(p, res.results, emaps)
